# revision 27
# baseline (speedup 1.0000x reference)
"""BP message passing on 8 trn2 NeuronCores.

Pipeline (all float math on device; host only shards/permutes index-selected
views of inputs/outputs between launches):
  L1 (factor-sharded): per-factor masked exp/marginalize/log -> lse, then
      damped factor->var messages fv_msg.            [dense]
  host: regroup fv_msg by variable (degree-class layout).
  L2 (var-sharded): segment sums per degree class -> var marginals,
      normalize, segment broadcast -> damped var->factor messages vf_msg.
      No collective needed: each core owns a var range. [dense]
  host: regroup vf_msg back to factor order.
  L3 (factor-sharded): factor marginals fm = N(v0+v1+v2), masked. [dense]
"""
import numpy as np

F = 1_000_000
E = 3_000_000
V = 500_000
NC = 8
P = 128
DAMP = 0.9
LN0 = -99.0

F_LOC = F // NC          # 125000
C1 = 977                 # ceil(125000/128)
FP = C1 * P              # 125056 padded factors per core
EP = 3 * FP              # 375168 padded edges per core
V_LOC = V // NC          # 62500

_CON = {}


def _concourse():
    if not _CON:
        import concourse.bass as bass
        import concourse.bacc as bacc
        import concourse.mybir as mybir
        import concourse.tile as tile
        from concourse.bass_utils import run_bass_kernel_spmd
        _CON.update(bass=bass, bacc=bacc, mybir=mybir, tile=tile,
                    run=run_bass_kernel_spmd)
    return _CON


def _chunks(total, step):
    out = []
    c = 0
    while c < total:
        out.append((c, min(step, total - c)))
        c += step
    return out


M89 = float(np.float32(np.float32(DAMP) * np.float32(LN0)))  # 0.9*-99 in f32


def build_l1(has_invalid):
    """fv = max(0.9*max(lse - vfp, -99) + 0.1*fvp, -99); outputs fv [128,3C1,2]."""
    C = _concourse()
    bass, mybir, tile, bacc = C["bass"], C["mybir"], C["tile"], C["bacc"]
    f32 = mybir.dt.float32
    nc = bacc.Bacc("TRN2", target_bir_lowering=False, debug=False, num_devices=NC)
    fpm = nc.dram_tensor("fpm", [P, C1, 8], f32, kind="ExternalInput").ap()
    vfp = nc.dram_tensor("vfp", [P, 3 * C1, 2], f32, kind="ExternalInput").ap()
    fvp = nc.dram_tensor("fvp", [P, 3 * C1, 2], f32, kind="ExternalInput").ap()
    if has_invalid:
        vld = nc.dram_tensor("vld", [P, C1, 8], f32, kind="ExternalInput").ap()
    fv = nc.dram_tensor("fv", [P, 3 * C1, 2], f32, kind="ExternalOutput").ap()
    from concourse.dve_ops import AFFINE_THEN_ADD
    CH = _chunks(C1, 326)
    with tile.TileContext(nc) as tc:
        with tc.tile_pool(name="io", bufs=2) as io, tc.tile_pool(name="wk", bufs=2) as wk, \
             tc.tile_pool(name="ps", bufs=1) as ps:
            st = ps.tile([P, C1, 3, 2], f32)
            # phase 1: all Exp + pair sums (ACT table: Exp only)
            for c0, w in CH:
                tf = io.tile([P, w, 8], f32, tag="tf")
                nc.sync.dma_start(out=tf[:], in_=fpm[:, c0:c0 + w, :])
                e8 = wk.tile([P, w, 8], f32, tag="e8")
                if has_invalid:
                    tvd = io.tile([P, w, 8], f32, tag="tvd")
                    nc.sync.dma_start(out=tvd[:], in_=vld[:, c0:c0 + w, :])
                    nc.vector.tensor_scalar_add(tf[:], tf[:], -LN0)
                    nc.vector.tensor_tensor(out=tf[:], in0=tf[:], in1=tvd[:], op=mybir.AluOpType.mult)
                    nc.scalar.activation(e8[:], tf[:], mybir.ActivationFunctionType.Exp, bias=LN0)
                else:
                    nc.scalar.activation(e8[:], tf[:], mybir.ActivationFunctionType.Exp)
                e8v = e8[:].rearrange("p w (a b) -> p w a b", b=2)          # [P,w,4,2]
                u = wk.tile([P, w, 4], f32, tag="u")
                nc.gpsimd.tensor_tensor(out=u[:], in0=e8v[:, :, :, 0], in1=e8v[:, :, :, 1], op=mybir.AluOpType.add)
                e8w = e8[:].rearrange("p w (a b c) -> p w a b c", b=2, c=2)  # [P,w,2,2,2]
                vv = wk.tile([P, w, 2, 2], f32, tag="vv")
                nc.vector.tensor_tensor(out=vv[:], in0=e8w[:, :, :, 0, :], in1=e8w[:, :, :, 1, :], op=mybir.AluOpType.add)
                uv = u[:].rearrange("p w (a b) -> p w a b", b=2)            # [P,w,2,2]
                sc = st[:, c0:c0 + w, :, :]
                nc.vector.tensor_tensor(out=sc[:, :, 0, :], in0=uv[:, :, :, 0], in1=uv[:, :, :, 1], op=mybir.AluOpType.add)
                nc.vector.tensor_tensor(out=sc[:, :, 1, :], in0=uv[:, :, 0, :], in1=uv[:, :, 1, :], op=mybir.AluOpType.add)
                nc.vector.tensor_tensor(out=sc[:, :, 2, :], in0=vv[:, :, 0, :], in1=vv[:, :, 1, :], op=mybir.AluOpType.add)
            # phase 2: all Ln + edge chain (ACT table: Ln only)
            for c0, w in CH:
                sc = st[:, c0:c0 + w, :, :]
                nc.scalar.activation(sc, sc, mybir.ActivationFunctionType.Ln)
                lsev = st[:, c0:c0 + w, :, :].rearrange("p w a b -> p (w a) b")
                tvf = io.tile([P, 3 * w, 2], f32, tag="tvf")
                tfv = io.tile([P, 3 * w, 2], f32, tag="tfv")
                nc.sync.dma_start(out=tvf[:], in_=vfp[:, 3 * c0:3 * (c0 + w), :])
                nc.sync.dma_start(out=tfv[:], in_=fvp[:, 3 * c0:3 * (c0 + w), :])
                t = wk.tile([P, 3 * w, 2], f32, tag="t")
                nc.vector.tensor_tensor(out=t[:], in0=lsev, in1=tvf[:], op=mybir.AluOpType.subtract)
                nc.vector.tensor_scalar(t[:], t[:], DAMP, M89, mybir.AluOpType.mult, mybir.AluOpType.max)
                tfl = t[:].rearrange("p e s -> p (e s)")
                ffl = tfv[:].rearrange("p e s -> p (e s)")
                nc.vector._custom_dve(AFFINE_THEN_ADD, out=tfl, in0=ffl, in1=tfl, s0=1.0 - DAMP, s1=0.0)
                nc.vector.tensor_scalar_max(t[:], t[:], LN0)
                nc.sync.dma_start(out=fv[:, 3 * c0:3 * (c0 + w), :], in_=t[:])
    nc.compile()
    return nc


def build_l2(classes, We, Wv, groups):
    """classes: list of (d, md, edge_col, vm_col); groups: list of
    (lo, hi, e0, e1, v0, v1) contiguous class runs, pipelined independently.
    ev/pv [P, We]; outputs vf [P, We], vm [P, Wv]."""
    C = _concourse()
    bass, mybir, tile, bacc = C["bass"], C["mybir"], C["tile"], C["bacc"]
    from concourse.dve_ops import AFFINE_THEN_ADD
    f32 = mybir.dt.float32
    nc = bacc.Bacc("TRN2", target_bir_lowering=False, debug=False, num_devices=NC)
    ev = nc.dram_tensor("ev", [P, max(We, 2)], f32, kind="ExternalInput").ap()
    pv = nc.dram_tensor("pv", [P, max(We, 2)], f32, kind="ExternalInput").ap()
    vf = nc.dram_tensor("vf", [P, max(We, 2)], f32, kind="ExternalOutput").ap()
    vm = nc.dram_tensor("vm", [P, Wv], f32, kind="ExternalOutput").ap()
    Mv = Wv // 2
    with tile.TileContext(nc) as tc:
        with tc.tile_pool(name="io", bufs=1) as io, tc.tile_pool(name="wk", bufs=1) as wk, \
             tc.tile_pool(name="tr", bufs=2) as tr, tc.tile_pool(name="nz", bufs=2) as nz:
            tev = io.tile([P, max(We, 2)], f32)
            tpv = io.tile([P, max(We, 2)], f32)
            tvm = wk.tile([P, Mv, 2], f32)
            big = wk.tile([P, max(We, 2)], f32)
            for lo, hi, e0, e1, v0, v1 in groups:
                if e1 > e0:
                    nc.sync.dma_start(out=tev[:, e0:e1], in_=ev[:, e0:e1])
                    nc.sync.dma_start(out=tpv[:, e0:e1], in_=pv[:, e0:e1])
                # segment sums for this group's classes
                for d, md, ec, vc in classes[lo:hi]:
                    vtgt = tvm[:, vc:vc + md, :]
                    if d == 0:
                        nc.vector.memset(vtgt, 0.0)
                        continue
                    blk = tev[:, ec:ec + md * d * 2].rearrange("p (m d s) -> p m d s", d=d, s=2)
                    if d == 1:
                        nc.gpsimd.tensor_copy(out=vtgt, in_=blk[:, :, 0, :])
                        continue
                    cur, cw = blk, d
                    while cw > 1:
                        h = (cw + 1) // 2
                        if cw == 2:
                            nc.vector.tensor_tensor(out=vtgt, in0=cur[:, :, 0, :], in1=cur[:, :, 1, :],
                                                    op=mybir.AluOpType.add)
                            break
                        nxt = tr.tile([P, md, h, 2], f32, tag=f"tr{d}")
                        ev2 = cur[:, :, 0:2 * (cw // 2), :].rearrange("p m (a b) s -> p m a b s", b=2)
                        nc.vector.tensor_tensor(out=nxt[:, :, 0:cw // 2, :], in0=ev2[:, :, :, 0, :],
                                                in1=ev2[:, :, :, 1, :], op=mybir.AluOpType.add)
                        if cw % 2:
                            nc.vector.tensor_copy(out=nxt[:, :, cw // 2, :], in_=cur[:, :, cw - 1, :])
                        cur, cw = nxt[:], h
                # normalize this group's tvm slice
                gm = v1 - v0
                tvs = tvm[:, v0:v1, :]
                mx = nz.tile([P, gm], f32, tag="mx")
                nc.vector.tensor_tensor(out=mx[:], in0=tvs[:, :, 0], in1=tvs[:, :, 1], op=mybir.AluOpType.max)
                sb = nz.tile([P, gm, 2], f32, tag="sb")
                nc.vector.tensor_tensor(out=sb[:], in0=tvs, in1=mx[:, :, None].to_broadcast([P, gm, 2]),
                                        op=mybir.AluOpType.subtract)
                nc.scalar.activation(sb[:], sb[:], mybir.ActivationFunctionType.Exp)
                ss = nz.tile([P, gm], f32, tag="ss")
                nc.vector.tensor_tensor(out=ss[:], in0=sb[:, :, 0], in1=sb[:, :, 1], op=mybir.AluOpType.add)
                nc.scalar.activation(ss[:], ss[:], mybir.ActivationFunctionType.Ln)
                nc.vector.tensor_tensor(out=mx[:], in0=mx[:], in1=ss[:], op=mybir.AluOpType.add)
                nc.vector.tensor_tensor(out=tvs, in0=tvs, in1=mx[:, :, None].to_broadcast([P, gm, 2]),
                                        op=mybir.AluOpType.subtract)
                nc.sync.dma_start(out=vm[:, 2 * v0:2 * v1], in_=tvs.rearrange("p m s -> p (m s)"))
                # segment-broadcast folded into the subtract (per-class bcast APs)
                for d, md, ec, vc in classes[lo:hi]:
                    if d == 0:
                        continue
                    dst = big[:, ec:ec + md * d * 2].rearrange("p (m d s) -> p m d s", d=d, s=2)
                    srcb = tvm[:, vc:vc + md, None, :].to_broadcast([P, md, d, 2])
                    blk = tev[:, ec:ec + md * d * 2].rearrange("p (m d s) -> p m d s", d=d, s=2)
                    nc.vector.tensor_tensor(out=dst, in0=srcb, in1=blk, op=mybir.AluOpType.subtract)
                if e1 > e0:
                    bs = big[:, e0:e1]
                    nc.vector.tensor_scalar(bs, bs, DAMP, M89, mybir.AluOpType.mult, mybir.AluOpType.max)
                    nc.vector._custom_dve(AFFINE_THEN_ADD, out=bs, in0=tpv[:, e0:e1], in1=bs,
                                          s0=1.0 - DAMP, s1=0.0)
                    nc.vector.tensor_scalar_max(bs, bs, LN0)
                    nc.sync.dma_start(out=vf[:, e0:e1], in_=bs)
    nc.compile()
    return nc


def build_l3(has_invalid):
    """fm8 = v0+v1+v2 over configs; fmn = fm8 - sum_p lse2(vp); masked."""
    C = _concourse()
    bass, mybir, tile, bacc = C["bass"], C["mybir"], C["tile"], C["bacc"]
    f32 = mybir.dt.float32
    nc = bacc.Bacc("TRN2", target_bir_lowering=False, debug=False, num_devices=NC)
    vfi = nc.dram_tensor("vfi", [P, 3 * C1, 2], f32, kind="ExternalInput").ap()
    if has_invalid:
        vld = nc.dram_tensor("vld", [P, C1, 8], f32, kind="ExternalInput").ap()
    fm = nc.dram_tensor("fm", [P, C1, 8], f32, kind="ExternalOutput").ap()
    CH = _chunks(C1, 245)
    with tile.TileContext(nc) as tc:
        with tc.tile_pool(name="io", bufs=2) as io, tc.tile_pool(name="wk", bufs=2) as wk, \
             tc.tile_pool(name="ps", bufs=1) as ps:
            tv = ps.tile([P, C1, 3, 2], f32)
            d3 = ps.tile([P, C1, 3], f32)
            m3 = ps.tile([P, C1, 3], f32)
            # phase 1: load + max/min/sub (DVE/GpSimd only)
            for c0, w in CH:
                nc.sync.dma_start(out=tv[:, c0:c0 + w, :, :],
                                  in_=vfi[:, 3 * c0:3 * (c0 + w), :].rearrange("p (w a) s -> p w a s", a=3))
                tc_ = tv[:, c0:c0 + w, :, :]
                mc, dc = m3[:, c0:c0 + w, :], d3[:, c0:c0 + w, :]
                nc.vector.tensor_tensor(out=mc, in0=tc_[:, :, :, 0], in1=tc_[:, :, :, 1], op=mybir.AluOpType.max)
                nc.vector.tensor_tensor(out=dc, in0=tc_[:, :, :, 0], in1=tc_[:, :, :, 1], op=mybir.AluOpType.min)
                nc.vector.tensor_tensor(out=dc, in0=dc, in1=mc, op=mybir.AluOpType.subtract)
            # phase 2: Exp over all chunks, then Ln(+1) over all chunks (coarse)
            CH2 = _chunks(C1, 489)
            for c0, w in CH2:
                nc.scalar.activation(d3[:, c0:c0 + w, :], d3[:, c0:c0 + w, :], mybir.ActivationFunctionType.Exp)
            for c0, w in CH2:
                nc.scalar.activation(d3[:, c0:c0 + w, :], d3[:, c0:c0 + w, :], mybir.ActivationFunctionType.Ln, bias=1.0)
            # phase 3: combine
            for c0, w in CH:
                tc_ = tv[:, c0:c0 + w, :, :]
                mc, dc = m3[:, c0:c0 + w, :], d3[:, c0:c0 + w, :]
                nc.gpsimd.tensor_tensor(out=mc, in0=mc, in1=dc, op=mybir.AluOpType.add)
                nn = wk.tile([P, w], f32, tag="nn")
                nc.gpsimd.tensor_tensor(out=nn[:], in0=mc[:, :, 0], in1=mc[:, :, 1], op=mybir.AluOpType.add)
                nc.gpsimd.tensor_tensor(out=nn[:], in0=nn[:], in1=mc[:, :, 2], op=mybir.AluOpType.add)
                v2s = wk.tile([P, w, 2], f32, tag="v2s")
                nc.vector.tensor_tensor(out=v2s[:], in0=tc_[:, :, 2, :], in1=nn[:, :, None].to_broadcast([P, w, 2]),
                                        op=mybir.AluOpType.subtract)
                t01 = wk.tile([P, w, 2, 2], f32, tag="t01")
                v0b = tc_[:, :, 0, :, None].to_broadcast([P, w, 2, 2])
                v1b = tc_[:, :, 1, None, :].to_broadcast([P, w, 2, 2])
                nc.vector.tensor_tensor(out=t01[:], in0=v0b, in1=v1b, op=mybir.AluOpType.add)
                fm8 = wk.tile([P, w, 8], f32, tag="fm8")
                fm8v = fm8[:].rearrange("p w (a b) -> p w a b", b=2)        # [P, w, 4, 2]
                t01b = t01[:].rearrange("p w a b -> p w (a b)")[:, :, :, None].to_broadcast([P, w, 4, 2])
                v2b = v2s[:, :, None, :].to_broadcast([P, w, 4, 2])
                nc.vector.tensor_tensor(out=fm8v, in0=t01b, in1=v2b, op=mybir.AluOpType.add)
                if has_invalid:
                    tvd = io.tile([P, w, 8], f32, tag="tvd")
                    nc.sync.dma_start(out=tvd[:], in_=vld[:, c0:c0 + w, :])
                    # out = fmn*valid + (-99)*(1-valid) = (fmn+99)*valid - 99
                    nc.vector.tensor_scalar_add(fm8[:], fm8[:], -LN0)
                    nc.vector.tensor_tensor(out=fm8[:], in0=fm8[:], in1=tvd[:], op=mybir.AluOpType.mult)
                    nc.vector.tensor_scalar_add(fm8[:], fm8[:], LN0)
                nc.sync.dma_start(out=fm[:, c0:c0 + w, :], in_=fm8[:])
    nc.compile()
    return nc


def _pad_rows(a, n):
    if a.shape[0] == n:
        return a
    out = np.zeros((n,) + a.shape[1:], a.dtype)
    out[: a.shape[0]] = a
    return out


def kernel(var_factor_prev_msg, factor_var_prev_msg, factor_prev_marginals,
           factor_var_adjacency, factor_var_indices, indexes_var_factor,
           factor_valid_configs, _timing=None):
    C = _concourse()
    run = C["run"]
    cores = list(range(NC))

    vfp = np.ascontiguousarray(np.asarray(var_factor_prev_msg, dtype=np.float32))
    fvp = np.ascontiguousarray(np.asarray(factor_var_prev_msg, dtype=np.float32))
    fpm = np.asarray(factor_prev_marginals, dtype=np.float32).reshape(F, 8)
    adj1 = np.asarray(factor_var_adjacency)[1].astype(np.int64)
    valid = np.asarray(factor_valid_configs).reshape(F, 8)
    has_invalid = bool((valid != 0).any())
    validf = (valid == 0).astype(np.float32) if has_invalid else None

    trace = _timing is not None
    times = []

    def _run(nc, maps):
        res = run(nc, maps, cores, trace=trace)
        if trace:
            times.append(res.exec_time_ns)
        return res.results

    # ---------- L1 ----------
    nc1 = build_l1(has_invalid)
    maps1 = []
    for i in range(NC):
        f0 = i * F_LOC
        m = {
            "fpm": _pad_rows(fpm[f0:f0 + F_LOC], FP).reshape(P, C1, 8),
            "vfp": _pad_rows(vfp[3 * f0:3 * (f0 + F_LOC)], EP).reshape(P, 3 * C1, 2),
            "fvp": _pad_rows(fvp[3 * f0:3 * (f0 + F_LOC)], EP).reshape(P, 3 * C1, 2),
        }
        if has_invalid:
            m["vld"] = _pad_rows(validf[f0:f0 + F_LOC], FP).reshape(P, C1, 8)
        maps1.append(m)
    res1 = _run(nc1, maps1)
    fv_full = np.concatenate(
        [res1[i]["fv"].reshape(EP, 2)[: 3 * F_LOC] for i in range(NC)], axis=0)

    # ---------- host: group edges by variable, degree classes ----------
    order = np.argsort(adj1, kind="stable")
    degree = np.bincount(adj1, minlength=V)
    cum = np.zeros(V + 1, np.int64)
    np.cumsum(degree, out=cum[1:])
    # per-core per-class var lists; degrees > DCAP merge into one padded class
    DCAP = 12
    dmax = int(degree.max()) if len(degree) else 0
    DBIG = max(dmax, DCAP + 1)     # pad width of the merged class
    core_class_vars = []           # [core][key] -> array of var ids
    for i in range(NC):
        v0, v1 = i * V_LOC, (i + 1) * V_LOC
        degs = degree[v0:v1]
        keys = np.minimum(degs, DCAP + 1)
        byd = {}
        sort_d = np.argsort(keys, kind="stable")
        bnd = np.searchsorted(keys[sort_d], np.arange(DCAP + 3))
        for k in range(0, DCAP + 2):
            ids = sort_d[bnd[k]:bnd[k + 1]]
            if len(ids):
                byd[k] = ids.astype(np.int64) + v0
        core_class_vars.append(byd)
    # global class shapes (key DCAP+1 -> device degree DBIG)
    class_md = {}
    for k in range(0, DCAP + 2):
        n = max(len(core_class_vars[i].get(k, ())) for i in range(NC))
        if n > 0:
            class_md[k] = (n + P - 1) // P
    classes = []           # (device_degree, md, edge_col, vm_col)
    class_keys = []
    We, Mv = 0, 0
    for k in sorted(class_md):
        md = class_md[k]
        d_dev = DBIG if k == DCAP + 1 else k
        classes.append((d_dev, md, We, Mv))
        class_keys.append(k)
        We += md * d_dev * 2
        Mv += md
    Wv = Mv * 2
    groups = []
    lo = 0
    acc = 0
    target = max(We // 6, 1)
    for idx, (d, md, ec, vc) in enumerate(classes):
        acc += md * d * 2
        last = idx == len(classes) - 1
        if acc >= target or last:
            hi = idx + 1
            e0 = classes[lo][2]
            e1 = We if last else classes[hi][2]
            v0 = classes[lo][3]
            v1 = Mv if last else classes[hi][3]
            groups.append((lo, hi, e0, e1, v0, v1))
            lo = hi
            acc = 0
    nc2 = build_l2(classes, We, Wv, groups)
    maps2 = []
    meta2 = []             # [core] -> list of (d, md, vids, eids)
    for i in range(NC):
        ev = np.zeros((P, max(We, 2)), np.float32)
        pvv = np.zeros((P, max(We, 2)), np.float32)
        meta = []
        for (d, md, ec, vc), k in zip(classes, class_keys):
            vids = core_class_vars[i].get(k, np.zeros(0, np.int64))
            if d == 0 or len(vids) == 0:
                meta.append((d, md, vids, None, None))
                continue
            pos = cum[vids][:, None] + np.arange(d)[None, :]          # [n,d]
            if k == DCAP + 1:
                msk = np.arange(d)[None, :] < degree[vids][:, None]
                eids = order[np.minimum(pos, cum[vids][:, None] + degree[vids][:, None] - 1)]
                vals = np.where(msk[:, :, None], fv_full[eids], 0.0).astype(np.float32)
                pvals = np.where(msk[:, :, None], vfp[eids], 0.0).astype(np.float32)
            else:
                msk = None
                eids = order[pos]
                vals = fv_full[eids]                                  # [n,d,2]
                pvals = vfp[eids]
            n_pad = P * md
            vals = _pad_rows(vals, n_pad).reshape(P, md, d, 2)
            pvals = _pad_rows(pvals, n_pad).reshape(P, md, d, 2)
            ev[:, ec:ec + md * d * 2] = vals.reshape(P, -1)
            pvv[:, ec:ec + md * d * 2] = pvals.reshape(P, -1)
            meta.append((d, md, vids, eids, msk))
        maps2.append({"ev": ev, "pv": pvv})
        meta2.append(meta)
    res2 = _run(nc2, maps2)

    var_marg = np.empty((V, 2), np.float32)
    vf_full = np.empty((E, 2), np.float32)
    for i in range(NC):
        vmo = res2[i]["vm"].reshape(P, Mv, 2)
        vfo = res2[i]["vf"].reshape(P, -1)
        for (d, md, vids, eids, msk), (d2, md2, ec, vc) in zip(meta2[i], classes):
            n = len(vids)
            if n:
                var_marg[vids] = vmo[:, vc:vc + md, :].reshape(P * md, 2)[:n]
            if d == 0 or n == 0:
                continue
            blk = vfo[:, ec:ec + md * d * 2].reshape(P * md, d, 2)[:n]
            if msk is None:
                vf_full[eids.reshape(-1)] = blk.reshape(-1, 2)
            else:
                vf_full[eids[msk]] = blk[msk]

    # ---------- L3 ----------
    nc3 = build_l3(has_invalid)
    maps3 = []
    for i in range(NC):
        f0 = i * F_LOC
        m = {"vfi": _pad_rows(vf_full[3 * f0:3 * (f0 + F_LOC)], EP).reshape(P, 3 * C1, 2)}
        if has_invalid:
            m["vld"] = maps1[i]["vld"]
        maps3.append(m)
    res3 = _run(nc3, maps3)
    fac_marg = np.concatenate(
        [res3[i]["fm"].reshape(FP, 8)[:F_LOC] for i in range(NC)], axis=0)

    if _timing is not None:
        _timing.extend(times)
    return (vf_full, fv_full, var_marg, fac_marg.reshape(F, 2, 2, 2))


# revision 28
# speedup vs baseline: 1.0137x; 1.0137x over previous
"""BP message passing on 8 trn2 NeuronCores.

Pipeline (all float math on device; host only shards/permutes index-selected
views of inputs/outputs between launches):
  L1 (factor-sharded): per-factor masked exp/marginalize/log -> lse, then
      damped factor->var messages fv_msg.            [dense]
  host: regroup fv_msg by variable (degree-class layout).
  L2 (var-sharded): segment sums per degree class -> var marginals,
      normalize, segment broadcast -> damped var->factor messages vf_msg.
      No collective needed: each core owns a var range. [dense]
  host: regroup vf_msg back to factor order.
  L3 (factor-sharded): factor marginals fm = N(v0+v1+v2), masked. [dense]
"""
import numpy as np

F = 1_000_000
E = 3_000_000
V = 500_000
NC = 8
P = 128
DAMP = 0.9
LN0 = -99.0

F_LOC = F // NC          # 125000
C1 = 977                 # ceil(125000/128)
FP = C1 * P              # 125056 padded factors per core
EP = 3 * FP              # 375168 padded edges per core
V_LOC = V // NC          # 62500

_CON = {}


def _concourse():
    if not _CON:
        import concourse.bass as bass
        import concourse.bacc as bacc
        import concourse.mybir as mybir
        import concourse.tile as tile
        from concourse.bass_utils import run_bass_kernel_spmd
        _CON.update(bass=bass, bacc=bacc, mybir=mybir, tile=tile,
                    run=run_bass_kernel_spmd)
    return _CON


def _chunks(total, step):
    out = []
    c = 0
    while c < total:
        out.append((c, min(step, total - c)))
        c += step
    return out


M89 = float(np.float32(np.float32(DAMP) * np.float32(LN0)))  # 0.9*-99 in f32


def build_l1(has_invalid):
    """fv = max(0.9*max(lse - vfp, -99) + 0.1*fvp, -99); outputs fv [128,3C1,2]."""
    C = _concourse()
    bass, mybir, tile, bacc = C["bass"], C["mybir"], C["tile"], C["bacc"]
    f32 = mybir.dt.float32
    nc = bacc.Bacc("TRN2", target_bir_lowering=False, debug=False, num_devices=NC)
    fpm = nc.dram_tensor("fpm", [P, C1, 8], f32, kind="ExternalInput").ap()
    vfp = nc.dram_tensor("vfp", [P, 3 * C1, 2], f32, kind="ExternalInput").ap()
    fvp = nc.dram_tensor("fvp", [P, 3 * C1, 2], f32, kind="ExternalInput").ap()
    if has_invalid:
        vld = nc.dram_tensor("vld", [P, C1, 8], f32, kind="ExternalInput").ap()
    fv = nc.dram_tensor("fv", [P, 3 * C1, 2], f32, kind="ExternalOutput").ap()
    from concourse.dve_ops import AFFINE_THEN_ADD
    CH = _chunks(C1, 326)
    with tile.TileContext(nc) as tc:
        with tc.tile_pool(name="io", bufs=2) as io, tc.tile_pool(name="wk", bufs=2) as wk, \
             tc.tile_pool(name="ps", bufs=1) as ps:
            st = ps.tile([P, C1, 3, 2], f32)
            # phase 1: all Exp + pair sums (ACT table: Exp only)
            for c0, w in CH:
                tf = io.tile([P, w, 8], f32, tag="tf")
                nc.sync.dma_start(out=tf[:], in_=fpm[:, c0:c0 + w, :])
                e8 = wk.tile([P, w, 8], f32, tag="e8")
                if has_invalid:
                    tvd = io.tile([P, w, 8], f32, tag="tvd")
                    nc.sync.dma_start(out=tvd[:], in_=vld[:, c0:c0 + w, :])
                    nc.vector.tensor_scalar_add(tf[:], tf[:], -LN0)
                    nc.vector.tensor_tensor(out=tf[:], in0=tf[:], in1=tvd[:], op=mybir.AluOpType.mult)
                    nc.scalar.activation(e8[:], tf[:], mybir.ActivationFunctionType.Exp, bias=LN0)
                else:
                    nc.scalar.activation(e8[:], tf[:], mybir.ActivationFunctionType.Exp)
                e8v = e8[:].rearrange("p w (a b) -> p w a b", b=2)          # [P,w,4,2]
                u = wk.tile([P, w, 4], f32, tag="u")
                nc.gpsimd.tensor_tensor(out=u[:], in0=e8v[:, :, :, 0], in1=e8v[:, :, :, 1], op=mybir.AluOpType.add)
                e8w = e8[:].rearrange("p w (a b c) -> p w a b c", b=2, c=2)  # [P,w,2,2,2]
                vv = wk.tile([P, w, 2, 2], f32, tag="vv")
                nc.vector.tensor_tensor(out=vv[:], in0=e8w[:, :, :, 0, :], in1=e8w[:, :, :, 1, :], op=mybir.AluOpType.add)
                uv = u[:].rearrange("p w (a b) -> p w a b", b=2)            # [P,w,2,2]
                sc = st[:, c0:c0 + w, :, :]
                nc.vector.tensor_tensor(out=sc[:, :, 0, :], in0=uv[:, :, :, 0], in1=uv[:, :, :, 1], op=mybir.AluOpType.add)
                nc.vector.tensor_tensor(out=sc[:, :, 1, :], in0=uv[:, :, 0, :], in1=uv[:, :, 1, :], op=mybir.AluOpType.add)
                nc.vector.tensor_tensor(out=sc[:, :, 2, :], in0=vv[:, :, 0, :], in1=vv[:, :, 1, :], op=mybir.AluOpType.add)
            # phase 2: all Ln + edge chain (ACT table: Ln only)
            for c0, w in CH:
                sc = st[:, c0:c0 + w, :, :]
                nc.scalar.activation(sc, sc, mybir.ActivationFunctionType.Ln)
                lsev = st[:, c0:c0 + w, :, :].rearrange("p w a b -> p (w a) b")
                tvf = io.tile([P, 3 * w, 2], f32, tag="tvf")
                tfv = io.tile([P, 3 * w, 2], f32, tag="tfv")
                nc.sync.dma_start(out=tvf[:], in_=vfp[:, 3 * c0:3 * (c0 + w), :])
                nc.sync.dma_start(out=tfv[:], in_=fvp[:, 3 * c0:3 * (c0 + w), :])
                t = wk.tile([P, 3 * w, 2], f32, tag="t")
                nc.vector.tensor_tensor(out=t[:], in0=lsev, in1=tvf[:], op=mybir.AluOpType.subtract)
                nc.vector.tensor_scalar(t[:], t[:], DAMP, M89, mybir.AluOpType.mult, mybir.AluOpType.max)
                tfl = t[:].rearrange("p e s -> p (e s)")
                ffl = tfv[:].rearrange("p e s -> p (e s)")
                nc.vector._custom_dve(AFFINE_THEN_ADD, out=tfl, in0=ffl, in1=tfl, s0=1.0 - DAMP, s1=0.0)
                nc.vector.tensor_scalar_max(t[:], t[:], LN0)
                nc.sync.dma_start(out=fv[:, 3 * c0:3 * (c0 + w), :], in_=t[:])
    nc.compile()
    return nc


def build_l2(classes, We, Wv, groups):
    """classes: list of (d, md, edge_col, vm_col); groups: list of
    (lo, hi, e0, e1, v0, v1) contiguous class runs, pipelined independently.
    ev/pv [P, We]; outputs vf [P, We], vm [P, Wv]."""
    C = _concourse()
    bass, mybir, tile, bacc = C["bass"], C["mybir"], C["tile"], C["bacc"]
    from concourse.dve_ops import AFFINE_THEN_ADD
    f32 = mybir.dt.float32
    nc = bacc.Bacc("TRN2", target_bir_lowering=False, debug=False, num_devices=NC)
    ev = nc.dram_tensor("ev", [P, max(We, 2)], f32, kind="ExternalInput").ap()
    pv = nc.dram_tensor("pv", [P, max(We, 2)], f32, kind="ExternalInput").ap()
    vf = nc.dram_tensor("vf", [P, max(We, 2)], f32, kind="ExternalOutput").ap()
    vm = nc.dram_tensor("vm", [P, Wv], f32, kind="ExternalOutput").ap()
    Mv = Wv // 2
    with tile.TileContext(nc) as tc:
        with tc.tile_pool(name="io", bufs=1) as io, tc.tile_pool(name="wk", bufs=1) as wk, \
             tc.tile_pool(name="tr", bufs=2) as tr, tc.tile_pool(name="nz", bufs=2) as nz:
            tev = io.tile([P, max(We, 2)], f32)
            tpv = io.tile([P, max(We, 2)], f32)
            tvm = wk.tile([P, Mv, 2], f32)
            big = wk.tile([P, max(We, 2)], f32)
            for lo, hi, e0, e1, v0, v1 in groups:
                if e1 > e0:
                    nc.sync.dma_start(out=tev[:, e0:e1], in_=ev[:, e0:e1])
                    nc.sync.dma_start(out=tpv[:, e0:e1], in_=pv[:, e0:e1])
                # segment sums for this group's classes
                for d, md, ec, vc in classes[lo:hi]:
                    vtgt = tvm[:, vc:vc + md, :]
                    if d == 0:
                        nc.vector.memset(vtgt, 0.0)
                        continue
                    blk = tev[:, ec:ec + md * d * 2].rearrange("p (m d s) -> p m d s", d=d, s=2)
                    if d == 1:
                        nc.gpsimd.tensor_copy(out=vtgt, in_=blk[:, :, 0, :])
                        continue
                    cur, cw = blk, d
                    while cw > 1:
                        h = (cw + 1) // 2
                        if cw == 2:
                            nc.vector.tensor_tensor(out=vtgt, in0=cur[:, :, 0, :], in1=cur[:, :, 1, :],
                                                    op=mybir.AluOpType.add)
                            break
                        nxt = tr.tile([P, md, h, 2], f32, tag=f"tr{d}")
                        ev2 = cur[:, :, 0:2 * (cw // 2), :].rearrange("p m (a b) s -> p m a b s", b=2)
                        nc.vector.tensor_tensor(out=nxt[:, :, 0:cw // 2, :], in0=ev2[:, :, :, 0, :],
                                                in1=ev2[:, :, :, 1, :], op=mybir.AluOpType.add)
                        if cw % 2:
                            nc.vector.tensor_copy(out=nxt[:, :, cw // 2, :], in_=cur[:, :, cw - 1, :])
                        cur, cw = nxt[:], h
                # normalize this group's tvm slice
                gm = v1 - v0
                tvs = tvm[:, v0:v1, :]
                mx = nz.tile([P, gm], f32, tag="mx")
                nc.vector.tensor_tensor(out=mx[:], in0=tvs[:, :, 0], in1=tvs[:, :, 1], op=mybir.AluOpType.max)
                sb = nz.tile([P, gm, 2], f32, tag="sb")
                nc.vector.tensor_tensor(out=sb[:], in0=tvs, in1=mx[:, :, None].to_broadcast([P, gm, 2]),
                                        op=mybir.AluOpType.subtract)
                nc.scalar.activation(sb[:], sb[:], mybir.ActivationFunctionType.Exp)
                ss = nz.tile([P, gm], f32, tag="ss")
                nc.vector.tensor_tensor(out=ss[:], in0=sb[:, :, 0], in1=sb[:, :, 1], op=mybir.AluOpType.add)
                nc.scalar.activation(ss[:], ss[:], mybir.ActivationFunctionType.Ln)
                nc.vector.tensor_tensor(out=mx[:], in0=mx[:], in1=ss[:], op=mybir.AluOpType.add)
                nc.vector.tensor_tensor(out=tvs, in0=tvs, in1=mx[:, :, None].to_broadcast([P, gm, 2]),
                                        op=mybir.AluOpType.subtract)
                nc.sync.dma_start(out=vm[:, 2 * v0:2 * v1], in_=tvs.rearrange("p m s -> p (m s)"))
                # segment-broadcast folded into the subtract (per-class bcast APs)
                for d, md, ec, vc in classes[lo:hi]:
                    if d == 0:
                        continue
                    dst = big[:, ec:ec + md * d * 2].rearrange("p (m d s) -> p m d s", d=d, s=2)
                    srcb = tvm[:, vc:vc + md, None, :].to_broadcast([P, md, d, 2])
                    blk = tev[:, ec:ec + md * d * 2].rearrange("p (m d s) -> p m d s", d=d, s=2)
                    nc.vector.tensor_tensor(out=dst, in0=srcb, in1=blk, op=mybir.AluOpType.subtract)
                if e1 > e0:
                    bs = big[:, e0:e1]
                    nc.vector.tensor_scalar(bs, bs, DAMP, M89, mybir.AluOpType.mult, mybir.AluOpType.max)
                    nc.vector._custom_dve(AFFINE_THEN_ADD, out=bs, in0=tpv[:, e0:e1], in1=bs,
                                          s0=1.0 - DAMP, s1=0.0)
                    nc.vector.tensor_scalar_max(bs, bs, LN0)
                    nc.sync.dma_start(out=vf[:, e0:e1], in_=bs)
    nc.compile()
    return nc


def build_l3(has_invalid):
    """fm8 = v0+v1+v2 over configs; fmn = fm8 - sum_p lse2(vp); masked."""
    C = _concourse()
    bass, mybir, tile, bacc = C["bass"], C["mybir"], C["tile"], C["bacc"]
    f32 = mybir.dt.float32
    nc = bacc.Bacc("TRN2", target_bir_lowering=False, debug=False, num_devices=NC)
    vfi = nc.dram_tensor("vfi", [P, 3 * C1, 2], f32, kind="ExternalInput").ap()
    if has_invalid:
        vld = nc.dram_tensor("vld", [P, C1, 8], f32, kind="ExternalInput").ap()
    fm = nc.dram_tensor("fm", [P, C1, 8], f32, kind="ExternalOutput").ap()
    CH = _chunks(C1, 245)
    with tile.TileContext(nc) as tc:
        with tc.tile_pool(name="io", bufs=2) as io, tc.tile_pool(name="wk", bufs=2) as wk, \
             tc.tile_pool(name="ps", bufs=1) as ps:
            tv = ps.tile([P, C1, 3, 2], f32)
            d3 = ps.tile([P, C1, 3], f32)
            m3 = ps.tile([P, C1, 3], f32)
            # phase 1: load + max/min/sub (DVE/GpSimd only)
            for c0, w in CH:
                nc.sync.dma_start(out=tv[:, c0:c0 + w, :, :],
                                  in_=vfi[:, 3 * c0:3 * (c0 + w), :].rearrange("p (w a) s -> p w a s", a=3))
                tc_ = tv[:, c0:c0 + w, :, :]
                mc, dc = m3[:, c0:c0 + w, :], d3[:, c0:c0 + w, :]
                nc.vector.tensor_tensor(out=mc, in0=tc_[:, :, :, 0], in1=tc_[:, :, :, 1], op=mybir.AluOpType.max)
                nc.vector.tensor_tensor(out=dc, in0=tc_[:, :, :, 0], in1=tc_[:, :, :, 1], op=mybir.AluOpType.min)
                nc.vector.tensor_tensor(out=dc, in0=dc, in1=mc, op=mybir.AluOpType.subtract)
            # phase 2: Exp over all chunks, then Ln(+1) over all chunks
            for c0, w in CH:
                nc.scalar.activation(d3[:, c0:c0 + w, :], d3[:, c0:c0 + w, :], mybir.ActivationFunctionType.Exp)
            for c0, w in CH:
                nc.scalar.activation(d3[:, c0:c0 + w, :], d3[:, c0:c0 + w, :], mybir.ActivationFunctionType.Ln, bias=1.0)
            # phase 3: combine
            for c0, w in CH:
                tc_ = tv[:, c0:c0 + w, :, :]
                mc, dc = m3[:, c0:c0 + w, :], d3[:, c0:c0 + w, :]
                nc.vector.tensor_tensor(out=mc, in0=mc, in1=dc, op=mybir.AluOpType.add)
                nn = wk.tile([P, w], f32, tag="nn")
                nc.vector.tensor_tensor(out=nn[:], in0=mc[:, :, 0], in1=mc[:, :, 1], op=mybir.AluOpType.add)
                nc.vector.tensor_tensor(out=nn[:], in0=nn[:], in1=mc[:, :, 2], op=mybir.AluOpType.add)
                v2s = wk.tile([P, w, 2], f32, tag="v2s")
                nc.vector.tensor_tensor(out=v2s[:], in0=tc_[:, :, 2, :], in1=nn[:, :, None].to_broadcast([P, w, 2]),
                                        op=mybir.AluOpType.subtract)
                t01 = wk.tile([P, w, 2, 2], f32, tag="t01")
                v0b = tc_[:, :, 0, :, None].to_broadcast([P, w, 2, 2])
                v1b = tc_[:, :, 1, None, :].to_broadcast([P, w, 2, 2])
                nc.vector.tensor_tensor(out=t01[:], in0=v0b, in1=v1b, op=mybir.AluOpType.add)
                fm8 = wk.tile([P, w, 8], f32, tag="fm8")
                fm8v = fm8[:].rearrange("p w (a b) -> p w a b", b=2)        # [P, w, 4, 2]
                t01b = t01[:].rearrange("p w a b -> p w (a b)")[:, :, :, None].to_broadcast([P, w, 4, 2])
                v2b = v2s[:, :, None, :].to_broadcast([P, w, 4, 2])
                nc.vector.tensor_tensor(out=fm8v, in0=t01b, in1=v2b, op=mybir.AluOpType.add)
                if has_invalid:
                    tvd = io.tile([P, w, 8], f32, tag="tvd")
                    nc.sync.dma_start(out=tvd[:], in_=vld[:, c0:c0 + w, :])
                    # out = fmn*valid + (-99)*(1-valid) = (fmn+99)*valid - 99
                    nc.vector.tensor_scalar_add(fm8[:], fm8[:], -LN0)
                    nc.vector.tensor_tensor(out=fm8[:], in0=fm8[:], in1=tvd[:], op=mybir.AluOpType.mult)
                    nc.vector.tensor_scalar_add(fm8[:], fm8[:], LN0)
                nc.sync.dma_start(out=fm[:, c0:c0 + w, :], in_=fm8[:])
    nc.compile()
    return nc


def _pad_rows(a, n):
    if a.shape[0] == n:
        return a
    out = np.zeros((n,) + a.shape[1:], a.dtype)
    out[: a.shape[0]] = a
    return out


def kernel(var_factor_prev_msg, factor_var_prev_msg, factor_prev_marginals,
           factor_var_adjacency, factor_var_indices, indexes_var_factor,
           factor_valid_configs, _timing=None):
    C = _concourse()
    run = C["run"]
    cores = list(range(NC))

    vfp = np.ascontiguousarray(np.asarray(var_factor_prev_msg, dtype=np.float32))
    fvp = np.ascontiguousarray(np.asarray(factor_var_prev_msg, dtype=np.float32))
    fpm = np.asarray(factor_prev_marginals, dtype=np.float32).reshape(F, 8)
    adj1 = np.asarray(factor_var_adjacency)[1].astype(np.int64)
    valid = np.asarray(factor_valid_configs).reshape(F, 8)
    has_invalid = bool((valid != 0).any())
    validf = (valid == 0).astype(np.float32) if has_invalid else None

    trace = _timing is not None
    times = []

    def _run(nc, maps):
        res = run(nc, maps, cores, trace=trace)
        if trace:
            times.append(res.exec_time_ns)
        return res.results

    # ---------- L1 ----------
    nc1 = build_l1(has_invalid)
    maps1 = []
    for i in range(NC):
        f0 = i * F_LOC
        m = {
            "fpm": _pad_rows(fpm[f0:f0 + F_LOC], FP).reshape(P, C1, 8),
            "vfp": _pad_rows(vfp[3 * f0:3 * (f0 + F_LOC)], EP).reshape(P, 3 * C1, 2),
            "fvp": _pad_rows(fvp[3 * f0:3 * (f0 + F_LOC)], EP).reshape(P, 3 * C1, 2),
        }
        if has_invalid:
            m["vld"] = _pad_rows(validf[f0:f0 + F_LOC], FP).reshape(P, C1, 8)
        maps1.append(m)
    res1 = _run(nc1, maps1)
    fv_full = np.concatenate(
        [res1[i]["fv"].reshape(EP, 2)[: 3 * F_LOC] for i in range(NC)], axis=0)

    # ---------- host: group edges by variable, degree classes ----------
    order = np.argsort(adj1, kind="stable")
    degree = np.bincount(adj1, minlength=V)
    cum = np.zeros(V + 1, np.int64)
    np.cumsum(degree, out=cum[1:])
    # per-core per-class var lists; degrees > DCAP merge into one padded class
    DCAP = 12
    dmax = int(degree.max()) if len(degree) else 0
    DBIG = max(dmax, DCAP + 1)     # pad width of the merged class
    core_class_vars = []           # [core][key] -> array of var ids
    for i in range(NC):
        v0, v1 = i * V_LOC, (i + 1) * V_LOC
        degs = degree[v0:v1]
        keys = np.minimum(degs, DCAP + 1)
        byd = {}
        sort_d = np.argsort(keys, kind="stable")
        bnd = np.searchsorted(keys[sort_d], np.arange(DCAP + 3))
        for k in range(0, DCAP + 2):
            ids = sort_d[bnd[k]:bnd[k + 1]]
            if len(ids):
                byd[k] = ids.astype(np.int64) + v0
        core_class_vars.append(byd)
    # global class shapes (key DCAP+1 -> device degree DBIG)
    class_md = {}
    for k in range(0, DCAP + 2):
        n = max(len(core_class_vars[i].get(k, ())) for i in range(NC))
        if n > 0:
            class_md[k] = (n + P - 1) // P
    classes = []           # (device_degree, md, edge_col, vm_col)
    class_keys = []
    We, Mv = 0, 0
    for k in sorted(class_md):
        md = class_md[k]
        d_dev = DBIG if k == DCAP + 1 else k
        classes.append((d_dev, md, We, Mv))
        class_keys.append(k)
        We += md * d_dev * 2
        Mv += md
    Wv = Mv * 2
    groups = []
    lo = 0
    acc = 0
    target = max(We // 6, 1)
    for idx, (d, md, ec, vc) in enumerate(classes):
        acc += md * d * 2
        last = idx == len(classes) - 1
        if acc >= target or last:
            hi = idx + 1
            e0 = classes[lo][2]
            e1 = We if last else classes[hi][2]
            v0 = classes[lo][3]
            v1 = Mv if last else classes[hi][3]
            groups.append((lo, hi, e0, e1, v0, v1))
            lo = hi
            acc = 0
    nc2 = build_l2(classes, We, Wv, groups)
    maps2 = []
    meta2 = []             # [core] -> list of (d, md, vids, eids)
    for i in range(NC):
        ev = np.zeros((P, max(We, 2)), np.float32)
        pvv = np.zeros((P, max(We, 2)), np.float32)
        meta = []
        for (d, md, ec, vc), k in zip(classes, class_keys):
            vids = core_class_vars[i].get(k, np.zeros(0, np.int64))
            if d == 0 or len(vids) == 0:
                meta.append((d, md, vids, None, None))
                continue
            pos = cum[vids][:, None] + np.arange(d)[None, :]          # [n,d]
            if k == DCAP + 1:
                msk = np.arange(d)[None, :] < degree[vids][:, None]
                eids = order[np.minimum(pos, cum[vids][:, None] + degree[vids][:, None] - 1)]
                vals = np.where(msk[:, :, None], fv_full[eids], 0.0).astype(np.float32)
                pvals = np.where(msk[:, :, None], vfp[eids], 0.0).astype(np.float32)
            else:
                msk = None
                eids = order[pos]
                vals = fv_full[eids]                                  # [n,d,2]
                pvals = vfp[eids]
            n_pad = P * md
            vals = _pad_rows(vals, n_pad).reshape(P, md, d, 2)
            pvals = _pad_rows(pvals, n_pad).reshape(P, md, d, 2)
            ev[:, ec:ec + md * d * 2] = vals.reshape(P, -1)
            pvv[:, ec:ec + md * d * 2] = pvals.reshape(P, -1)
            meta.append((d, md, vids, eids, msk))
        maps2.append({"ev": ev, "pv": pvv})
        meta2.append(meta)
    res2 = _run(nc2, maps2)

    var_marg = np.empty((V, 2), np.float32)
    vf_full = np.empty((E, 2), np.float32)
    for i in range(NC):
        vmo = res2[i]["vm"].reshape(P, Mv, 2)
        vfo = res2[i]["vf"].reshape(P, -1)
        for (d, md, vids, eids, msk), (d2, md2, ec, vc) in zip(meta2[i], classes):
            n = len(vids)
            if n:
                var_marg[vids] = vmo[:, vc:vc + md, :].reshape(P * md, 2)[:n]
            if d == 0 or n == 0:
                continue
            blk = vfo[:, ec:ec + md * d * 2].reshape(P * md, d, 2)[:n]
            if msk is None:
                vf_full[eids.reshape(-1)] = blk.reshape(-1, 2)
            else:
                vf_full[eids[msk]] = blk[msk]

    # ---------- L3 ----------
    nc3 = build_l3(has_invalid)
    maps3 = []
    for i in range(NC):
        f0 = i * F_LOC
        m = {"vfi": _pad_rows(vf_full[3 * f0:3 * (f0 + F_LOC)], EP).reshape(P, 3 * C1, 2)}
        if has_invalid:
            m["vld"] = maps1[i]["vld"]
        maps3.append(m)
    res3 = _run(nc3, maps3)
    fac_marg = np.concatenate(
        [res3[i]["fm"].reshape(FP, 8)[:F_LOC] for i in range(NC)], axis=0)

    if _timing is not None:
        _timing.extend(times)
    return (vf_full, fv_full, var_marg, fac_marg.reshape(F, 2, 2, 2))


# revision 29
# speedup vs baseline: 1.0644x; 1.0500x over previous
"""BP message passing on 8 trn2 NeuronCores.

Pipeline (all float math on device; host only shards/permutes index-selected
views of inputs/outputs between launches):
  L1 (factor-sharded): per-factor masked exp/marginalize/log -> lse, then
      damped factor->var messages fv_msg.            [dense]
  host: regroup fv_msg by variable (degree-class layout).
  L2 (var-sharded): segment sums per degree class -> var marginals,
      normalize, segment broadcast -> damped var->factor messages vf_msg.
      No collective needed: each core owns a var range. [dense]
  host: regroup vf_msg back to factor order.
  L3 (factor-sharded): factor marginals fm = N(v0+v1+v2), masked. [dense]
"""
import numpy as np

F = 1_000_000
E = 3_000_000
V = 500_000
NC = 8
P = 128
DAMP = 0.9
LN0 = -99.0

F_LOC = F // NC          # 125000
C1 = 977                 # ceil(125000/128)
FP = C1 * P              # 125056 padded factors per core
EP = 3 * FP              # 375168 padded edges per core
V_LOC = V // NC          # 62500

_CON = {}


def _concourse():
    if not _CON:
        import concourse.bass as bass
        import concourse.bacc as bacc
        import concourse.mybir as mybir
        import concourse.tile as tile
        from concourse.bass_utils import run_bass_kernel_spmd
        _CON.update(bass=bass, bacc=bacc, mybir=mybir, tile=tile,
                    run=run_bass_kernel_spmd)
    return _CON


def _chunks(total, step):
    out = []
    c = 0
    while c < total:
        out.append((c, min(step, total - c)))
        c += step
    return out


M89 = float(np.float32(np.float32(DAMP) * np.float32(LN0)))  # 0.9*-99 in f32


def build_l1(has_invalid):
    """fv = max(0.9*max(lse - vfp, -99) + 0.1*fvp, -99); outputs fv [128,3C1,2]."""
    C = _concourse()
    bass, mybir, tile, bacc = C["bass"], C["mybir"], C["tile"], C["bacc"]
    f32 = mybir.dt.float32
    nc = bacc.Bacc("TRN2", target_bir_lowering=False, debug=False, num_devices=NC)
    fpm = nc.dram_tensor("fpm", [P, C1, 8], f32, kind="ExternalInput").ap()
    vfp = nc.dram_tensor("vfp", [P, 3 * C1, 2], f32, kind="ExternalInput").ap()
    fvp = nc.dram_tensor("fvp", [P, 3 * C1, 2], f32, kind="ExternalInput").ap()
    if has_invalid:
        vld = nc.dram_tensor("vld", [P, C1, 8], f32, kind="ExternalInput").ap()
    fv = nc.dram_tensor("fv", [P, 3 * C1, 2], f32, kind="ExternalOutput").ap()
    from concourse.dve_ops import AFFINE_THEN_ADD
    CH = _chunks(C1, 326)
    with tile.TileContext(nc) as tc:
        with tc.tile_pool(name="io", bufs=2) as io, tc.tile_pool(name="wk", bufs=2) as wk, \
             tc.tile_pool(name="ps", bufs=1) as ps:
            st = ps.tile([P, C1, 3, 2], f32)
            # phase 1: all Exp + pair sums (ACT table: Exp only)
            for c0, w in CH:
                tf = io.tile([P, w, 8], f32, tag="tf")
                nc.sync.dma_start(out=tf[:], in_=fpm[:, c0:c0 + w, :])
                e8 = wk.tile([P, w, 8], f32, tag="e8")
                if has_invalid:
                    tvd = io.tile([P, w, 8], f32, tag="tvd")
                    nc.sync.dma_start(out=tvd[:], in_=vld[:, c0:c0 + w, :])
                    nc.vector.tensor_scalar_add(tf[:], tf[:], -LN0)
                    nc.vector.tensor_tensor(out=tf[:], in0=tf[:], in1=tvd[:], op=mybir.AluOpType.mult)
                    nc.scalar.activation(e8[:], tf[:], mybir.ActivationFunctionType.Exp, bias=LN0)
                else:
                    nc.scalar.activation(e8[:], tf[:], mybir.ActivationFunctionType.Exp)
                e8v = e8[:].rearrange("p w (a b) -> p w a b", b=2)          # [P,w,4,2]
                u = wk.tile([P, w, 4], f32, tag="u")
                nc.gpsimd.tensor_tensor(out=u[:], in0=e8v[:, :, :, 0], in1=e8v[:, :, :, 1], op=mybir.AluOpType.add)
                e8w = e8[:].rearrange("p w (a b c) -> p w a b c", b=2, c=2)  # [P,w,2,2,2]
                vv = wk.tile([P, w, 2, 2], f32, tag="vv")
                nc.vector.tensor_tensor(out=vv[:], in0=e8w[:, :, :, 0, :], in1=e8w[:, :, :, 1, :], op=mybir.AluOpType.add)
                uv = u[:].rearrange("p w (a b) -> p w a b", b=2)            # [P,w,2,2]
                sc = st[:, c0:c0 + w, :, :]
                nc.vector.tensor_tensor(out=sc[:, :, 0, :], in0=uv[:, :, :, 0], in1=uv[:, :, :, 1], op=mybir.AluOpType.add)
                nc.vector.tensor_tensor(out=sc[:, :, 1, :], in0=uv[:, :, 0, :], in1=uv[:, :, 1, :], op=mybir.AluOpType.add)
                nc.vector.tensor_tensor(out=sc[:, :, 2, :], in0=vv[:, :, 0, :], in1=vv[:, :, 1, :], op=mybir.AluOpType.add)
            # phase 2: all Ln + edge chain (ACT table: Ln only)
            for c0, w in CH:
                sc = st[:, c0:c0 + w, :, :]
                nc.scalar.activation(sc, sc, mybir.ActivationFunctionType.Ln)
                lsev = st[:, c0:c0 + w, :, :].rearrange("p w a b -> p (w a) b")
                tvf = io.tile([P, 3 * w, 2], f32, tag="tvf")
                tfv = io.tile([P, 3 * w, 2], f32, tag="tfv")
                nc.sync.dma_start(out=tvf[:], in_=vfp[:, 3 * c0:3 * (c0 + w), :])
                nc.sync.dma_start(out=tfv[:], in_=fvp[:, 3 * c0:3 * (c0 + w), :])
                t = wk.tile([P, 3 * w, 2], f32, tag="t")
                nc.vector.tensor_tensor(out=t[:], in0=lsev, in1=tvf[:], op=mybir.AluOpType.subtract)
                nc.vector.tensor_scalar(t[:], t[:], DAMP, M89, mybir.AluOpType.mult, mybir.AluOpType.max)
                tfl = t[:].rearrange("p e s -> p (e s)")
                ffl = tfv[:].rearrange("p e s -> p (e s)")
                nc.vector._custom_dve(AFFINE_THEN_ADD, out=tfl, in0=ffl, in1=tfl, s0=1.0 - DAMP, s1=0.0)
                nc.vector.tensor_scalar_max(t[:], t[:], LN0)
                nc.sync.dma_start(out=fv[:, 3 * c0:3 * (c0 + w), :], in_=t[:])
    nc.compile()
    return nc


def build_l2(classes, We, Wv, groups):
    """classes: list of (d, md, edge_col, vm_col); groups: list of
    (lo, hi, e0, e1, v0, v1) contiguous class runs, pipelined independently.
    ev/pv [P, We]; outputs vf [P, We], vm [P, Wv]."""
    C = _concourse()
    bass, mybir, tile, bacc = C["bass"], C["mybir"], C["tile"], C["bacc"]
    from concourse.dve_ops import AFFINE_THEN_ADD
    f32 = mybir.dt.float32
    nc = bacc.Bacc("TRN2", target_bir_lowering=False, debug=False, num_devices=NC)
    ev = nc.dram_tensor("ev", [P, max(We, 2)], f32, kind="ExternalInput").ap()
    pv = nc.dram_tensor("pv", [P, max(We, 2)], f32, kind="ExternalInput").ap()
    vf = nc.dram_tensor("vf", [P, max(We, 2)], f32, kind="ExternalOutput").ap()
    vm = nc.dram_tensor("vm", [P, Wv], f32, kind="ExternalOutput").ap()
    Mv = Wv // 2
    with tile.TileContext(nc) as tc:
        with tc.tile_pool(name="io", bufs=1) as io, tc.tile_pool(name="wk", bufs=1) as wk, \
             tc.tile_pool(name="tr", bufs=2) as tr, tc.tile_pool(name="nz", bufs=2) as nz:
            tev = io.tile([P, max(We, 2)], f32)
            tpv = io.tile([P, max(We, 2)], f32)
            tvm = wk.tile([P, Mv, 2], f32)
            big = wk.tile([P, max(We, 2)], f32)
            sbs = {}
            mxs = {}
            # phase A per group: DMA, trees, pre-normalize sub, Exp
            for gi, (lo, hi, e0, e1, v0, v1) in enumerate(groups):
                if e1 > e0:
                    nc.sync.dma_start(out=tev[:, e0:e1], in_=ev[:, e0:e1])
                    nc.sync.dma_start(out=tpv[:, e0:e1], in_=pv[:, e0:e1])
                for d, md, ec, vc in classes[lo:hi]:
                    vtgt = tvm[:, vc:vc + md, :]
                    if d == 0:
                        nc.vector.memset(vtgt, 0.0)
                        continue
                    blk = tev[:, ec:ec + md * d * 2].rearrange("p (m d s) -> p m d s", d=d, s=2)
                    if d == 1:
                        nc.gpsimd.tensor_copy(out=vtgt, in_=blk[:, :, 0, :])
                        continue
                    cur, cw = blk, d
                    while cw > 1:
                        h = (cw + 1) // 2
                        if cw == 2:
                            nc.vector.tensor_tensor(out=vtgt, in0=cur[:, :, 0, :], in1=cur[:, :, 1, :],
                                                    op=mybir.AluOpType.add)
                            break
                        nxt = tr.tile([P, md, h, 2], f32, tag=f"tr{d}")
                        ev2 = cur[:, :, 0:2 * (cw // 2), :].rearrange("p m (a b) s -> p m a b s", b=2)
                        nc.vector.tensor_tensor(out=nxt[:, :, 0:cw // 2, :], in0=ev2[:, :, :, 0, :],
                                                in1=ev2[:, :, :, 1, :], op=mybir.AluOpType.add)
                        if cw % 2:
                            nc.vector.tensor_copy(out=nxt[:, :, cw // 2, :], in_=cur[:, :, cw - 1, :])
                        cur, cw = nxt[:], h
                gm = v1 - v0
                tvs = tvm[:, v0:v1, :]
                mx = nz.tile([P, gm], f32, tag=f"mx{gi}")
                nc.vector.tensor_tensor(out=mx[:], in0=tvs[:, :, 0], in1=tvs[:, :, 1], op=mybir.AluOpType.max)
                sb = nz.tile([P, gm, 2], f32, tag=f"sb{gi}")
                nc.vector.tensor_tensor(out=sb[:], in0=tvs, in1=mx[:, :, None].to_broadcast([P, gm, 2]),
                                        op=mybir.AluOpType.subtract)
                nc.scalar.activation(sb[:], sb[:], mybir.ActivationFunctionType.Exp)
                sbs[gi], mxs[gi] = sb, mx
            # phase B per group: Ln, finish normalize, fused vf chain, outputs
            for gi, (lo, hi, e0, e1, v0, v1) in enumerate(groups):
                gm = v1 - v0
                tvs = tvm[:, v0:v1, :]
                sb, mx = sbs[gi], mxs[gi]
                ss = nz.tile([P, gm], f32, tag=f"ss{gi}")
                nc.vector.tensor_tensor(out=ss[:], in0=sb[:, :, 0], in1=sb[:, :, 1], op=mybir.AluOpType.add)
                nc.scalar.activation(ss[:], ss[:], mybir.ActivationFunctionType.Ln)
                nc.vector.tensor_tensor(out=mx[:], in0=mx[:], in1=ss[:], op=mybir.AluOpType.add)
                nc.vector.tensor_tensor(out=tvs, in0=tvs, in1=mx[:, :, None].to_broadcast([P, gm, 2]),
                                        op=mybir.AluOpType.subtract)
                nc.sync.dma_start(out=vm[:, 2 * v0:2 * v1], in_=tvs.rearrange("p m s -> p (m s)"))
                for d, md, ec, vc in classes[lo:hi]:
                    if d == 0:
                        continue
                    dst = big[:, ec:ec + md * d * 2].rearrange("p (m d s) -> p m d s", d=d, s=2)
                    srcb = tvm[:, vc:vc + md, None, :].to_broadcast([P, md, d, 2])
                    blk = tev[:, ec:ec + md * d * 2].rearrange("p (m d s) -> p m d s", d=d, s=2)
                    nc.vector.tensor_tensor(out=dst, in0=srcb, in1=blk, op=mybir.AluOpType.subtract)
                if e1 > e0:
                    bs = big[:, e0:e1]
                    nc.vector.tensor_scalar(bs, bs, DAMP, M89, mybir.AluOpType.mult, mybir.AluOpType.max)
                    nc.vector._custom_dve(AFFINE_THEN_ADD, out=bs, in0=tpv[:, e0:e1], in1=bs,
                                          s0=1.0 - DAMP, s1=0.0)
                    nc.vector.tensor_scalar_max(bs, bs, LN0)
                    nc.sync.dma_start(out=vf[:, e0:e1], in_=bs)
    nc.compile()
    return nc


def build_l3(has_invalid):
    """fm8 = v0+v1+v2 over configs; fmn = fm8 - sum_p lse2(vp); masked."""
    C = _concourse()
    bass, mybir, tile, bacc = C["bass"], C["mybir"], C["tile"], C["bacc"]
    f32 = mybir.dt.float32
    nc = bacc.Bacc("TRN2", target_bir_lowering=False, debug=False, num_devices=NC)
    vfi = nc.dram_tensor("vfi", [P, 3 * C1, 2], f32, kind="ExternalInput").ap()
    if has_invalid:
        vld = nc.dram_tensor("vld", [P, C1, 8], f32, kind="ExternalInput").ap()
    fm = nc.dram_tensor("fm", [P, C1, 8], f32, kind="ExternalOutput").ap()
    CH = _chunks(C1, 245)
    with tile.TileContext(nc) as tc:
        with tc.tile_pool(name="io", bufs=2) as io, tc.tile_pool(name="wk", bufs=2) as wk, \
             tc.tile_pool(name="ps", bufs=1) as ps:
            tv = ps.tile([P, C1, 3, 2], f32)
            d3 = ps.tile([P, C1, 3], f32)
            m3 = ps.tile([P, C1, 3], f32)
            # phase 1: load + max/min/sub (DVE/GpSimd only)
            for c0, w in CH:
                nc.sync.dma_start(out=tv[:, c0:c0 + w, :, :],
                                  in_=vfi[:, 3 * c0:3 * (c0 + w), :].rearrange("p (w a) s -> p w a s", a=3))
                tc_ = tv[:, c0:c0 + w, :, :]
                mc, dc = m3[:, c0:c0 + w, :], d3[:, c0:c0 + w, :]
                nc.vector.tensor_tensor(out=mc, in0=tc_[:, :, :, 0], in1=tc_[:, :, :, 1], op=mybir.AluOpType.max)
                nc.vector.tensor_tensor(out=dc, in0=tc_[:, :, :, 0], in1=tc_[:, :, :, 1], op=mybir.AluOpType.min)
                nc.vector.tensor_tensor(out=dc, in0=dc, in1=mc, op=mybir.AluOpType.subtract)
            # phase 2: Exp over all chunks, then Ln(+1) over all chunks
            for c0, w in CH:
                nc.scalar.activation(d3[:, c0:c0 + w, :], d3[:, c0:c0 + w, :], mybir.ActivationFunctionType.Exp)
            for c0, w in CH:
                nc.scalar.activation(d3[:, c0:c0 + w, :], d3[:, c0:c0 + w, :], mybir.ActivationFunctionType.Ln, bias=1.0)
            # phase 3: combine
            for c0, w in CH:
                tc_ = tv[:, c0:c0 + w, :, :]
                mc, dc = m3[:, c0:c0 + w, :], d3[:, c0:c0 + w, :]
                nc.vector.tensor_tensor(out=mc, in0=mc, in1=dc, op=mybir.AluOpType.add)
                nn = wk.tile([P, w], f32, tag="nn")
                nc.vector.tensor_tensor(out=nn[:], in0=mc[:, :, 0], in1=mc[:, :, 1], op=mybir.AluOpType.add)
                nc.vector.tensor_tensor(out=nn[:], in0=nn[:], in1=mc[:, :, 2], op=mybir.AluOpType.add)
                v2s = wk.tile([P, w, 2], f32, tag="v2s")
                nc.vector.tensor_tensor(out=v2s[:], in0=tc_[:, :, 2, :], in1=nn[:, :, None].to_broadcast([P, w, 2]),
                                        op=mybir.AluOpType.subtract)
                t01 = wk.tile([P, w, 2, 2], f32, tag="t01")
                v0b = tc_[:, :, 0, :, None].to_broadcast([P, w, 2, 2])
                v1b = tc_[:, :, 1, None, :].to_broadcast([P, w, 2, 2])
                nc.vector.tensor_tensor(out=t01[:], in0=v0b, in1=v1b, op=mybir.AluOpType.add)
                fm8 = wk.tile([P, w, 8], f32, tag="fm8")
                fm8v = fm8[:].rearrange("p w (a b) -> p w a b", b=2)        # [P, w, 4, 2]
                t01b = t01[:].rearrange("p w a b -> p w (a b)")[:, :, :, None].to_broadcast([P, w, 4, 2])
                v2b = v2s[:, :, None, :].to_broadcast([P, w, 4, 2])
                nc.vector.tensor_tensor(out=fm8v, in0=t01b, in1=v2b, op=mybir.AluOpType.add)
                if has_invalid:
                    tvd = io.tile([P, w, 8], f32, tag="tvd")
                    nc.sync.dma_start(out=tvd[:], in_=vld[:, c0:c0 + w, :])
                    # out = fmn*valid + (-99)*(1-valid) = (fmn+99)*valid - 99
                    nc.vector.tensor_scalar_add(fm8[:], fm8[:], -LN0)
                    nc.vector.tensor_tensor(out=fm8[:], in0=fm8[:], in1=tvd[:], op=mybir.AluOpType.mult)
                    nc.vector.tensor_scalar_add(fm8[:], fm8[:], LN0)
                nc.sync.dma_start(out=fm[:, c0:c0 + w, :], in_=fm8[:])
    nc.compile()
    return nc


def _pad_rows(a, n):
    if a.shape[0] == n:
        return a
    out = np.zeros((n,) + a.shape[1:], a.dtype)
    out[: a.shape[0]] = a
    return out


def kernel(var_factor_prev_msg, factor_var_prev_msg, factor_prev_marginals,
           factor_var_adjacency, factor_var_indices, indexes_var_factor,
           factor_valid_configs, _timing=None):
    C = _concourse()
    run = C["run"]
    cores = list(range(NC))

    vfp = np.ascontiguousarray(np.asarray(var_factor_prev_msg, dtype=np.float32))
    fvp = np.ascontiguousarray(np.asarray(factor_var_prev_msg, dtype=np.float32))
    fpm = np.asarray(factor_prev_marginals, dtype=np.float32).reshape(F, 8)
    adj1 = np.asarray(factor_var_adjacency)[1].astype(np.int64)
    valid = np.asarray(factor_valid_configs).reshape(F, 8)
    has_invalid = bool((valid != 0).any())
    validf = (valid == 0).astype(np.float32) if has_invalid else None

    trace = _timing is not None
    times = []

    def _run(nc, maps):
        res = run(nc, maps, cores, trace=trace)
        if trace:
            times.append(res.exec_time_ns)
        return res.results

    # ---------- L1 ----------
    nc1 = build_l1(has_invalid)
    maps1 = []
    for i in range(NC):
        f0 = i * F_LOC
        m = {
            "fpm": _pad_rows(fpm[f0:f0 + F_LOC], FP).reshape(P, C1, 8),
            "vfp": _pad_rows(vfp[3 * f0:3 * (f0 + F_LOC)], EP).reshape(P, 3 * C1, 2),
            "fvp": _pad_rows(fvp[3 * f0:3 * (f0 + F_LOC)], EP).reshape(P, 3 * C1, 2),
        }
        if has_invalid:
            m["vld"] = _pad_rows(validf[f0:f0 + F_LOC], FP).reshape(P, C1, 8)
        maps1.append(m)
    res1 = _run(nc1, maps1)
    fv_full = np.concatenate(
        [res1[i]["fv"].reshape(EP, 2)[: 3 * F_LOC] for i in range(NC)], axis=0)

    # ---------- host: group edges by variable, degree classes ----------
    order = np.argsort(adj1, kind="stable")
    degree = np.bincount(adj1, minlength=V)
    cum = np.zeros(V + 1, np.int64)
    np.cumsum(degree, out=cum[1:])
    # per-core per-class var lists; degrees > DCAP merge into one padded class
    DCAP = 12
    dmax = int(degree.max()) if len(degree) else 0
    DBIG = max(dmax, DCAP + 1)     # pad width of the merged class
    core_class_vars = []           # [core][key] -> array of var ids
    for i in range(NC):
        v0, v1 = i * V_LOC, (i + 1) * V_LOC
        degs = degree[v0:v1]
        keys = np.minimum(degs, DCAP + 1)
        byd = {}
        sort_d = np.argsort(keys, kind="stable")
        bnd = np.searchsorted(keys[sort_d], np.arange(DCAP + 3))
        for k in range(0, DCAP + 2):
            ids = sort_d[bnd[k]:bnd[k + 1]]
            if len(ids):
                byd[k] = ids.astype(np.int64) + v0
        core_class_vars.append(byd)
    # global class shapes (key DCAP+1 -> device degree DBIG)
    class_md = {}
    for k in range(0, DCAP + 2):
        n = max(len(core_class_vars[i].get(k, ())) for i in range(NC))
        if n > 0:
            class_md[k] = (n + P - 1) // P
    classes = []           # (device_degree, md, edge_col, vm_col)
    class_keys = []
    We, Mv = 0, 0
    for k in sorted(class_md):
        md = class_md[k]
        d_dev = DBIG if k == DCAP + 1 else k
        classes.append((d_dev, md, We, Mv))
        class_keys.append(k)
        We += md * d_dev * 2
        Mv += md
    Wv = Mv * 2
    groups = []
    lo = 0
    acc = 0
    target = max(We // 6, 1)
    for idx, (d, md, ec, vc) in enumerate(classes):
        acc += md * d * 2
        last = idx == len(classes) - 1
        if acc >= target or last:
            hi = idx + 1
            e0 = classes[lo][2]
            e1 = We if last else classes[hi][2]
            v0 = classes[lo][3]
            v1 = Mv if last else classes[hi][3]
            groups.append((lo, hi, e0, e1, v0, v1))
            lo = hi
            acc = 0
    nc2 = build_l2(classes, We, Wv, groups)
    maps2 = []
    meta2 = []             # [core] -> list of (d, md, vids, eids)
    for i in range(NC):
        ev = np.zeros((P, max(We, 2)), np.float32)
        pvv = np.zeros((P, max(We, 2)), np.float32)
        meta = []
        for (d, md, ec, vc), k in zip(classes, class_keys):
            vids = core_class_vars[i].get(k, np.zeros(0, np.int64))
            if d == 0 or len(vids) == 0:
                meta.append((d, md, vids, None, None))
                continue
            pos = cum[vids][:, None] + np.arange(d)[None, :]          # [n,d]
            if k == DCAP + 1:
                msk = np.arange(d)[None, :] < degree[vids][:, None]
                eids = order[np.minimum(pos, cum[vids][:, None] + degree[vids][:, None] - 1)]
                vals = np.where(msk[:, :, None], fv_full[eids], 0.0).astype(np.float32)
                pvals = np.where(msk[:, :, None], vfp[eids], 0.0).astype(np.float32)
            else:
                msk = None
                eids = order[pos]
                vals = fv_full[eids]                                  # [n,d,2]
                pvals = vfp[eids]
            n_pad = P * md
            vals = _pad_rows(vals, n_pad).reshape(P, md, d, 2)
            pvals = _pad_rows(pvals, n_pad).reshape(P, md, d, 2)
            ev[:, ec:ec + md * d * 2] = vals.reshape(P, -1)
            pvv[:, ec:ec + md * d * 2] = pvals.reshape(P, -1)
            meta.append((d, md, vids, eids, msk))
        maps2.append({"ev": ev, "pv": pvv})
        meta2.append(meta)
    res2 = _run(nc2, maps2)

    var_marg = np.empty((V, 2), np.float32)
    vf_full = np.empty((E, 2), np.float32)
    for i in range(NC):
        vmo = res2[i]["vm"].reshape(P, Mv, 2)
        vfo = res2[i]["vf"].reshape(P, -1)
        for (d, md, vids, eids, msk), (d2, md2, ec, vc) in zip(meta2[i], classes):
            n = len(vids)
            if n:
                var_marg[vids] = vmo[:, vc:vc + md, :].reshape(P * md, 2)[:n]
            if d == 0 or n == 0:
                continue
            blk = vfo[:, ec:ec + md * d * 2].reshape(P * md, d, 2)[:n]
            if msk is None:
                vf_full[eids.reshape(-1)] = blk.reshape(-1, 2)
            else:
                vf_full[eids[msk]] = blk[msk]

    # ---------- L3 ----------
    nc3 = build_l3(has_invalid)
    maps3 = []
    for i in range(NC):
        f0 = i * F_LOC
        m = {"vfi": _pad_rows(vf_full[3 * f0:3 * (f0 + F_LOC)], EP).reshape(P, 3 * C1, 2)}
        if has_invalid:
            m["vld"] = maps1[i]["vld"]
        maps3.append(m)
    res3 = _run(nc3, maps3)
    fac_marg = np.concatenate(
        [res3[i]["fm"].reshape(FP, 8)[:F_LOC] for i in range(NC)], axis=0)

    if _timing is not None:
        _timing.extend(times)
    return (vf_full, fv_full, var_marg, fac_marg.reshape(F, 2, 2, 2))
